# revision 1
# baseline (speedup 1.0000x reference)
"""Trainium2 Bass kernel for a dense transformer block (B=8, N=1024, C=768, H=12).

Sharding: data-parallel over batch -- one batch element per NeuronCore (8 cores),
weights replicated, no collectives.

Per-core dataflow (x_b: [1024, 768]):
  LN1 (token-major, gain/bias folded into qkv_w on host)
  -> transpose to feature-major hT [768, 1024]
  -> QKV: q,k feature-major [64, 1024]/head; v token-major (+ ones column)
  -> per head: scores^T [keys, queries] = kT.T @ qT (scale folded into Wq),
     + rel-bias Toeplitz band (host-precomputed, bf16), exp (no max-sub; scores
     are small by construction), AV matmul with ones column -> unnormalized
     attn out (feature-major) + softmax sums; normalize via reciprocal +
     PE broadcast + multiply
  -> proj (feature-major) -> transpose -> +x residual (in-place) -> LN2
  -> transpose -> MLP fc1+gelu / fc2 (feature-major) -> transpose -> +residual.
"""

import os

import numpy as np

B, N, C, H, D = 8, 1024, 768, 12, 64
NT = N // 128   # 8 token tiles
KT = C // 128   # 6 feature tiles
F1 = 4 * C      # 3072
RT = F1 // 128  # 24
W = 2 * N - 1   # 2047 toeplitz band width
EPS = 1e-5

LAST_RESULTS = None  # stash of the last BassKernelResults (for test.py)

_NC_CACHE = {}


def _build_nc(reps=1):
    from contextlib import ExitStack

    import concourse.bacc as bacc
    import concourse.tile as tile
    from concourse import masks, mybir

    f32 = mybir.dt.float32
    f32r = mybir.dt.float32r
    bf16 = mybir.dt.bfloat16

    def R(ap):
        return ap.bitcast(f32r)
    AF = mybir.ActivationFunctionType
    AX = mybir.AxisListType
    OP = mybir.AluOpType

    nc = bacc.Bacc(
        "TRN2",
        target_bir_lowering=False,
        debug=False,
        enable_asserts=False,
        num_devices=8,
    )

    x_d = nc.dram_tensor("x", [N, C], f32, kind="ExternalInput").ap()
    wqkv_d = nc.dram_tensor("wqkvT", [C, 3 * C], f32, kind="ExternalInput").ap()
    bqkv_d = nc.dram_tensor("bqkv", [1, 3 * C], f32, kind="ExternalInput").ap()
    wproj_d = nc.dram_tensor("wprojT", [C, C], f32, kind="ExternalInput").ap()
    bproj_d = nc.dram_tensor("bproj", [1, C], f32, kind="ExternalInput").ap()
    wfc1_d = nc.dram_tensor("wfc1t", [RT, 128, C], f32, kind="ExternalInput").ap()
    bfc1_d = nc.dram_tensor("bfc1", [1, F1], f32, kind="ExternalInput").ap()
    wfc2_d = nc.dram_tensor("wfc2T", [F1, C], f32, kind="ExternalInput").ap()
    bfc2_d = nc.dram_tensor("bfc2", [1, C], f32, kind="ExternalInput").ap()
    rb_d = nc.dram_tensor("rband", [H, 128, W], bf16, kind="ExternalInput").ap()
    sel_d = nc.dram_tensor("sel", [2, 128], f32, kind="ExternalInput").ap()
    out_d = nc.dram_tensor("out", [N, C], f32, kind="ExternalOutput").ap()

    with tile.TileContext(nc) as tc, ExitStack() as ctx:
        # ---------------- kernel-wide pools (opened first, closed last: LIFO ok)
        cpool = ctx.enter_context(tc.tile_pool(name="const", bufs=1))
        ident = cpool.tile([128, 128], f32, tag="ident")
        masks.make_identity(nc, ident[:])
        identb = cpool.tile([128, 128], bf16, tag="identb")
        masks.make_identity(nc, identb[:])
        onesP = cpool.tile([1, 128], f32, tag="onesP")
        nc.any.memset(onesP[:], 1.0)
        sel = cpool.tile([2, 128], f32, tag="sel")
        nc.sync.dma_start(sel[:], sel_d[:])
        epsc = cpool.tile([128, 1], f32, tag="eps")
        nc.any.memset(epsc[:], EPS)
        bqkv_sb = cpool.tile([128, 18], f32, tag="bqkv")
        nc.sync.dma_start(bqkv_sb[:], bqkv_d[0].rearrange("(a p) -> p a", p=128))
        bv_row = cpool.tile([1, C], f32, tag="bvrow")
        nc.sync.dma_start(bv_row[:], bqkv_d[:, 2 * C :])
        bproj_sb = cpool.tile([128, 6], f32, tag="bproj")
        nc.sync.dma_start(bproj_sb[:], bproj_d[0].rearrange("(a p) -> p a", p=128))
        bfc1_sb = cpool.tile([128, RT], f32, tag="bfc1")
        nc.sync.dma_start(bfc1_sb[:], bfc1_d[0].rearrange("(a p) -> p a", p=128))
        bfc2_sb = cpool.tile([128, 6], f32, tag="bfc2")
        nc.sync.dma_start(bfc2_sb[:], bfc2_d[0].rearrange("(a p) -> p a", p=128))

        stat = ctx.enter_context(tc.tile_pool(name="stat", bufs=8))
        # chain pool: big buffers with slot-cycling via shared tags
        chain = ctx.enter_context(tc.tile_pool(name="chain", bufs=1))

        def fm_tile(name, dt=f32):
            return chain.tile([128, N], dt, tag="fm1024", bufs=19, name=name)

        def layernorm(dst_ap, src_ap, scratch_ap):
            """dst = (src - mean(src)) * rsqrt(var(src) + eps); scratch may alias dst."""
            sums = stat.tile([128, 1], f32, tag="sums", name="sums")
            nc.vector.reduce_sum(sums[:], src_ap, axis=AX.X)
            mu = stat.tile([128, 1], f32, tag="mu", name="mu")
            nc.vector.tensor_scalar_mul(mu[:], sums[:], 1.0 / C)
            nc.vector.tensor_mul(scratch_ap, src_ap, src_ap)
            ssq = stat.tile([128, 1], f32, tag="ssq", name="ssq")
            nc.vector.reduce_sum(ssq[:], scratch_ap, axis=AX.X)
            musq = stat.tile([128, 1], f32, tag="musq", name="musq")
            nc.vector.tensor_mul(musq[:], mu[:], mu[:])
            var = stat.tile([128, 1], f32, tag="var", name="var")
            nc.vector.tensor_scalar(
                var[:], ssq[:], 1.0 / C, musq[:], op0=OP.mult, op1=OP.subtract
            )
            sd = stat.tile([128, 1], f32, tag="sd", name="sd")
            nc.scalar.activation(sd[:], var[:], AF.Sqrt, bias=epsc[:])
            rstd = stat.tile([128, 1], f32, tag="rstd", name="rstd")
            nc.vector.reciprocal(rstd[:], sd[:])
            nmr = stat.tile([128, 1], f32, tag="nmr", name="nmr")
            nc.vector.tensor_scalar(
                nmr[:], mu[:], rstd[:], -1.0, op0=OP.mult, op1=OP.mult
            )
            nc.vector.tensor_scalar(
                dst_ap, src_ap, rstd[:], nmr[:], op0=OP.mult, op1=OP.add
            )

        for _rep in range(reps):
            # persistent per-batch state
            xs = [chain.tile([128, C], f32, tag="x", bufs=NT, name=f"x{t}") for t in range(NT)]
            hT = [fm_tile(f"hT{i}", f32r) for i in range(KT)]
            vaug = [
                chain.tile([128, H * 65], bf16, tag="vaug", bufs=NT, name=f"vaug{t}")
                for t in range(NT)
            ]
            s_all = chain.tile([H, N], f32, tag="sall", bufs=1, name="sall")

            # ---------------- phase A+B: load x, LN1, transpose -> hT
            with tc.tile_pool(name="psB", bufs=6, space="PSUM") as psB:
                for t in range(NT):
                    nc.sync.dma_start(xs[t][:], x_d[t * 128 : (t + 1) * 128, :])
                    h1 = chain.tile([128, C], f32, tag="hln", bufs=5, name=f"h1_{t}")
                    layernorm(h1[:], xs[t][:], h1[:])
                    for ct in range(KT):
                        ps = psB.tile([128, 128], f32, tag="tp", name="psb")
                        nc.tensor.transpose(
                            ps[:], h1[:, ct * 128 : (ct + 1) * 128], ident[:]
                        )
                        nc.any.tensor_copy(hT[ct][:, t * 128 : (t + 1) * 128], ps[:])

            # ---------------- phase C: QKV
            qkT = [fm_tile(f"qkT{i}", f32r) for i in range(12)]
            with tc.tile_pool(name="wqkv", bufs=KT) as wq_pool:
                wq = []
                for ct in range(KT):
                    wt = wq_pool.tile([128, 3 * C], f32r, tag="wq", name=f"wq{ct}")
                    for wdc in range(3):
                        nc.sync.dma_start(
                            wt[:, wdc * 768 : (wdc + 1) * 768],
                            wqkv_d[
                                ct * 128 : (ct + 1) * 128, wdc * 768 : (wdc + 1) * 768
                            ].bitcast(f32r),
                        )
                    wq.append(wt)
                with tc.tile_pool(name="psC", bufs=3, space="PSUM") as psC:
                    # q,k feature-major
                    for jt in range(12):
                        for qc in range(2):
                            ps = psC.tile([128, 512], f32, tag="ps", name="psc")
                            for ct in range(KT):
                                nc.tensor.matmul(
                                    ps[:],
                                    wq[ct][:, jt * 128 : (jt + 1) * 128],
                                    hT[ct][:, qc * 512 : (qc + 1) * 512],
                                    start=(ct == 0),
                                    stop=(ct == KT - 1),
                                )
                            nc.vector.tensor_scalar_add(
                                qkT[jt][:, qc * 512 : (qc + 1) * 512],
                                ps[:],
                                bqkv_sb[:, jt : jt + 1],
                            )
                    # v token-major, bias via rank-1 ones matmul, ones col for sums
                    for t in range(NT):
                        vview = vaug[t][:].rearrange("p (h e) -> p h e", e=65)
                        for vc in range(2):
                            ps = psC.tile([128, 384], f32, tag="psv", bufs=2, name="psv")
                            for ct in range(KT):
                                nc.tensor.matmul(
                                    ps[:],
                                    hT[ct][:, t * 128 : (t + 1) * 128],
                                    wq[ct][:, 2 * C + vc * 384 : 2 * C + (vc + 1) * 384],
                                    start=(ct == 0),
                                    stop=False,
                                )
                            nc.tensor.matmul(
                                ps[:],
                                onesP[:],
                                bv_row[:, vc * 384 : (vc + 1) * 384],
                                start=False,
                                stop=True,
                            )
                            nc.vector.tensor_copy(
                                vview[:, vc * 6 : (vc + 1) * 6, 0:64],
                                ps[:].rearrange("p (h e) -> p h e", e=64),
                            )
                        nc.any.memset(vview[:, :, 64:65], 1.0)

            # ---------------- phase D: attention
            aT = [fm_tile(f"aT{i}", f32r) for i in range(KT)]
            with (
                tc.tile_pool(name="rbp", bufs=4) as rbp,
                tc.tile_pool(name="ptp", bufs=18) as ptp,
                tc.tile_pool(name="srowp", bufs=4) as srowp,
                tc.tile_pool(name="oddp", bufs=3) as oddp,
                tc.tile_pool(name="psS", bufs=3, space="PSUM") as psS,
                tc.tile_pool(name="psAV", bufs=2, space="PSUM") as psAV,
            ):
                for hp in range(KT):
                    # head pair (2hp, 2hp+1): even head at partitions 0:64, odd at
                    # 64:128 of the same qkT tiles -> score matmuls of the pair
                    # dispatch adjacently into disjoint PE row-groups (HW overlap)
                    rbs = []
                    for odd in range(2):
                        rb = rbp.tile([128, W], bf16, tag="rb", name=f"rb{2*hp+odd}")
                        nc.sync.dma_start(rb[:], rb_d[2 * hp + odd])
                        rbs.append(rb)
                    ptiles = [[], []]
                    for kc in range(NT):
                        pss = []
                        for odd in range(2):
                            ro = odd * 64
                            ps = psS.tile([128, 1024], f32, tag="ps", name="pss")
                            for qc in range(2):
                                nc.tensor.matmul(
                                    ps[:, qc * 512 : (qc + 1) * 512],
                                    qkT[6 + hp][ro : ro + 64, kc * 128 : (kc + 1) * 128],
                                    qkT[hp][ro : ro + 64, qc * 512 : (qc + 1) * 512],
                                    start=True,
                                    stop=False,
                                )
                            pss.append(ps)
                        for odd in range(2):
                            ps = pss[odd]
                            for qc in range(2):
                                off = 1023 - kc * 128 + qc * 512
                                nc.tensor.matmul(
                                    ps[:, qc * 512 : (qc + 1) * 512],
                                    identb[:],
                                    rbs[odd][:, off : off + 512],
                                    start=False,
                                    stop=True,
                                )
                            pt = ptp.tile([128, 1024], bf16, tag="pt", name="pt")
                            nc.scalar.activation(pt[:], ps[:], AF.Exp)
                            ptiles[odd].append(pt)
                    for odd in range(2):
                        h = 2 * hp + odd
                        for qc in range(2):
                            pav = psAV.tile([128, 512], f32, tag="pav", name="pav")
                            for kc in range(NT):
                                nc.tensor.matmul(
                                    pav[0:65, :],
                                    vaug[kc][:, h * 65 : (h + 1) * 65],
                                    ptiles[odd][kc][:, qc * 512 : (qc + 1) * 512],
                                    start=(kc == 0),
                                    stop=(kc == NT - 1),
                                )
                            if odd:
                                tmp = oddp.tile(
                                    [128, 512], f32r, tag="odd", name="avodd"
                                )
                                nc.vector.tensor_copy(tmp[0:64, :], pav[0:64, :])
                                nc.sync.dma_start(
                                    aT[hp][64:128, qc * 512 : (qc + 1) * 512],
                                    tmp[0:64, :],
                                )
                            else:
                                nc.vector.tensor_copy(
                                    aT[hp][0:64, qc * 512 : (qc + 1) * 512],
                                    pav[0:64, :],
                                )
                            srow = srowp.tile([128, 512], f32, tag="srow", name="srow")
                            nc.vector.tensor_copy(srow[64:65, :], pav[64:65, :])
                            nc.sync.dma_start(
                                s_all[h : h + 1, qc * 512 : (qc + 1) * 512],
                                srow[64:65, :],
                            )

            # ---------------- normalize + proj, fused per qc half
            yT = [fm_tile(f"yT{i}") for i in range(KT)]
            with tc.tile_pool(name="wpp", bufs=KT) as wpp:
                wp = []
                for c in range(KT):
                    wt = wpp.tile([128, C], f32r, tag="wp", name=f"wp{c}")
                    nc.sync.dma_start(wt[:], wproj_d[c * 128 : (c + 1) * 128, :].bitcast(f32r))
                    wp.append(wt)
                with (
                    tc.tile_pool(name="stgp", bufs=6) as stgp,
                    tc.tile_pool(name="psE", bufs=KT, space="PSUM") as psE,
                    tc.tile_pool(name="psNorm", bufs=2, space="PSUM") as psN,
                ):
                    nc.vector.reciprocal_approx_fast(s_all[:], s_all[:])
                    for qc in range(2):
                        pse = [
                            psE.tile([128, 512], f32, tag="pse", name=f"pse{qc}_{co}")
                            for co in range(KT)
                        ]
                        for hp in range(KT):
                            st2 = stgp.tile([2, 512], f32, tag="stg", name="st2")
                            nc.sync.dma_start(
                                st2[:],
                                s_all[2 * hp : 2 * hp + 2, qc * 512 : (qc + 1) * 512],
                            )
                            psb = psN.tile([128, 512], f32, tag="psn", name="psn")
                            nc.tensor.matmul(
                                psb[:], sel[:], st2[:], start=True, stop=True
                            )
                            nc.vector.tensor_mul(
                                aT[hp][:, qc * 512 : (qc + 1) * 512],
                                aT[hp][:, qc * 512 : (qc + 1) * 512],
                                psb[:],
                            )
                            for co in range(KT):
                                nc.tensor.matmul(
                                    pse[co][:],
                                    wp[hp][:, co * 128 : (co + 1) * 128],
                                    aT[hp][:, qc * 512 : (qc + 1) * 512],
                                    start=(hp == 0),
                                    stop=(hp == KT - 1),
                                )
                        for co in range(KT):
                            nc.vector.tensor_scalar_add(
                                yT[co][:, qc * 512 : (qc + 1) * 512],
                                pse[co][:],
                                bproj_sb[:, co : co + 1],
                            )

            # ---------------- phase F: transpose y, residual in-place, LN2, -> h2T
            h2T = [fm_tile(f"h2T{i}", f32r) for i in range(KT)]
            with tc.tile_pool(name="psF", bufs=6, space="PSUM") as psF:
                for t in range(NT):
                    for ct in range(KT):
                        ps = psF.tile([128, 128], f32, tag="tp", name="psf")
                        nc.tensor.transpose(
                            ps[:], yT[ct][:, t * 128 : (t + 1) * 128], ident[:]
                        )
                        nc.vector.tensor_add(
                            xs[t][:, ct * 128 : (ct + 1) * 128],
                            xs[t][:, ct * 128 : (ct + 1) * 128],
                            ps[:],
                        )
                    h2 = chain.tile([128, C], f32, tag="hln", bufs=5, name=f"h2_{t}")
                    layernorm(h2[:], xs[t][:], h2[:])
                    for ct in range(KT):
                        ps = psF.tile([128, 128], f32, tag="tp", name="psf2")
                        nc.tensor.transpose(
                            ps[:], h2[:, ct * 128 : (ct + 1) * 128], ident[:]
                        )
                        nc.any.tensor_copy(h2T[ct][:, t * 128 : (t + 1) * 128], ps[:])

            # ---------------- phase H: MLP + final residual + store
            with (
                tc.tile_pool(name="w1p", bufs=4) as w1p,
                tc.tile_pool(name="w2p", bufs=4) as w2p,
                tc.tile_pool(name="grp", bufs=4) as grp,
                tc.tile_pool(name="o2p", bufs=7) as o2p,
                tc.tile_pool(name="obp", bufs=3) as obp,
            ):
                for qc in range(2):
                    with tc.tile_pool(name="psO", bufs=6, space="PSUM") as ps_o:
                        pso = [
                            ps_o.tile([128, 512], f32, tag="pso", name=f"pso{qc}_{i}")
                            for i in range(KT)
                        ]
                        with tc.tile_pool(name="psG2", bufs=2, space="PSUM") as ps_g:
                            for r in range(RT):
                                w1 = w1p.tile([128, C], f32r, tag="w1", name=f"w1_{r}")
                                nc.sync.dma_start(w1[:], wfc1_d[r].bitcast(f32r))
                                w2 = w2p.tile([128, C], f32r, tag="w2", name=f"w2_{r}")
                                nc.sync.dma_start(
                                    w2[:],
                                    wfc2_d[r * 128 : (r + 1) * 128, :].bitcast(f32r),
                                )
                                psg = ps_g.tile([128, 512], f32, tag="psg", name="psg")
                                for ct in range(KT):
                                    nc.tensor.matmul(
                                        psg[:],
                                        w1[:, ct * 128 : (ct + 1) * 128],
                                        h2T[ct][:, qc * 512 : (qc + 1) * 512],
                                        start=(ct == 0),
                                        stop=(ct == KT - 1),
                                    )
                                gr = grp.tile([128, 512], f32r, tag="gr", name="gr")
                                nc.scalar.activation(
                                    gr[:], psg[:], AF.Gelu, bias=bfc1_sb[:, r : r + 1]
                                )
                                for co in range(KT):
                                    nc.tensor.matmul(
                                        pso[co][:],
                                        w2[:, co * 128 : (co + 1) * 128],
                                        gr[:],
                                        start=(r == 0),
                                        stop=(r == RT - 1),
                                    )
                        o2 = []
                        for co in range(KT):
                            o2t = o2p.tile([128, 512], f32, tag="o2", name=f"o2_{qc}_{co}")
                            nc.vector.tensor_scalar_add(
                                o2t[:], pso[co][:], bfc2_sb[:, co : co + 1]
                            )
                            o2.append(o2t)
                    with tc.tile_pool(name="psH", bufs=2, space="PSUM") as psH:
                        for t4 in range(4):
                            t = qc * 4 + t4
                            ob = obp.tile([128, C], f32, tag="ob", name="ob")
                            for co in range(KT):
                                ps = psH.tile([128, 128], f32, tag="tp", name="psh")
                                nc.tensor.transpose(
                                    ps[:], o2[co][:, t4 * 128 : (t4 + 1) * 128], ident[:]
                                )
                                nc.vector.tensor_add(
                                    ob[:, co * 128 : (co + 1) * 128],
                                    xs[t][:, co * 128 : (co + 1) * 128],
                                    ps[:],
                                )
                            nc.sync.dma_start(out_d[t * 128 : (t + 1) * 128, :], ob[:])

    nc.compile()
    return nc


def _get_nc(reps=1):
    key = f"nc{reps}"
    if key not in _NC_CACHE:
        _NC_CACHE[key] = _build_nc(reps)
    return _NC_CACHE[key]


def _host_prep(inputs):
    import ml_dtypes

    inp = {k: np.asarray(v) for k, v in inputs.items()}
    x = np.ascontiguousarray(inp["x"], dtype=np.float32)  # [8, 1024, 768]
    g1 = inp["ln1_g"].astype(np.float64)
    b1 = inp["ln1_b"].astype(np.float64)
    qkv_w = inp["qkv_w"].astype(np.float64)  # [2304, 768]
    Ws = qkv_w.copy()
    Ws[:C] *= D ** (-0.5)  # fold attention scale into Wq
    wqkvT = np.ascontiguousarray((Ws * g1[None, :]).T).astype(np.float32)  # [768, 2304]
    bqkv = (Ws @ b1).astype(np.float32).reshape(1, 3 * C)

    wprojT = np.ascontiguousarray(inp["proj_w"].astype(np.float32).T)  # [768, 768]
    bproj = inp["proj_b"].astype(np.float32).reshape(1, C)

    g2 = inp["ln2_g"].astype(np.float64)
    b2 = inp["ln2_b"].astype(np.float64)
    fc1_w = inp["fc1_w"].astype(np.float64)  # [3072, 768]
    wfc1T = (fc1_w * g2[None, :]).T.astype(np.float32)  # [768, 3072]
    # pre-tiled: wfc1t[r, p, ct*128+j] = wfc1T[ct*128+p, r*128+j]
    wfc1t = np.ascontiguousarray(
        wfc1T.reshape(KT, 128, RT, 128).transpose(2, 1, 0, 3).reshape(RT, 128, C)
    )
    bfc1 = (fc1_w @ b2 + inp["fc1_b"].astype(np.float64)).astype(np.float32)
    bfc1 = bfc1.reshape(1, F1)
    wfc2T = np.ascontiguousarray(inp["fc2_w"].astype(np.float32).T)  # [3072, 768]
    bfc2 = inp["fc2_b"].astype(np.float32).reshape(1, C)

    # rel-bias toeplitz band: rband[h, p, w] = rel_table[clip(p + 1087 - w, 0, 128), h]
    tab = inp["rel_table"].astype(np.float32)  # [129, 12]
    p_i = np.arange(128)
    w_i = np.arange(W)
    idx = np.clip(p_i[:, None] + (N + 63) - w_i[None, :], 0, 2 * 64)
    rband = np.ascontiguousarray(tab[idx, :].transpose(2, 0, 1)).astype(
        ml_dtypes.bfloat16
    )  # [12, 128, 2047]

    sel = np.zeros((2, 128), np.float32)
    sel[0, 0:64] = 1.0
    sel[1, 64:128] = 1.0
    shared = {
        "sel": sel,
        "wqkvT": wqkvT,
        "bqkv": bqkv,
        "wprojT": wprojT,
        "bproj": bproj,
        "wfc1t": wfc1t,
        "bfc1": bfc1,
        "wfc2T": wfc2T,
        "bfc2": bfc2,
        "rband": rband,
    }
    in_maps = [{"x": np.ascontiguousarray(x[c]), **shared} for c in range(B)]
    return in_maps


def _make_runner(reps=1):
    import jax
    from jax.experimental.shard_map import shard_map
    from jax.sharding import Mesh, NamedSharding, PartitionSpec

    from concourse import bass2jax, mybir

    nc = _get_nc(reps)
    bass2jax.install_neuronx_cc_hook()

    partition_name = nc.partition_id_tensor.name if nc.partition_id_tensor else None
    in_names, out_names, out_avals, zero_outs = [], [], [], []
    for alloc in nc.m.functions[0].allocations:
        if not isinstance(alloc, mybir.MemoryLocationSet):
            continue
        name = alloc.memorylocations[0].name
        if alloc.kind == "ExternalInput":
            if name != partition_name:
                in_names.append(name)
        elif alloc.kind == "ExternalOutput":
            out_names.append(name)
            shape = tuple(alloc.tensor_shape)
            dtype = mybir.dt.np(alloc.dtype)
            out_avals.append(jax.core.ShapedArray(shape, dtype))
            zero_outs.append(np.zeros(shape, dtype))
    n_params = len(in_names)
    all_names = tuple(in_names) + tuple(out_names)
    if partition_name is not None:
        all_names = all_names + (partition_name,)
    donate = tuple(range(n_params, n_params + len(out_names)))

    def _body(*args):
        operands = list(args)
        if partition_name is not None:
            operands.append(bass2jax.partition_id_tensor())
        outs = bass2jax._bass_exec_p.bind(
            *operands,
            out_avals=tuple(out_avals),
            in_names=all_names,
            out_names=tuple(out_names),
            lowering_input_output_aliases=(),
            sim_require_finite=True,
            sim_require_nnan=True,
            nc=nc,
        )
        return tuple(outs)

    def _body_k(k):
        def body(*args):
            ins = list(args[:n_params])
            outs = list(args[n_params:])
            for _ in range(k):
                outs = list(_body(*ins, *outs))
            return tuple(outs)

        return body

    devices = jax.devices()[:B]
    mesh = Mesh(np.asarray(devices), ("core",))
    in_specs = (PartitionSpec("core"),) * (n_params + len(out_names))
    out_specs = (PartitionSpec("core"),) * len(out_names)

    def make_fn(k):
        return jax.jit(
            shard_map(
                _body_k(k),
                mesh=mesh,
                in_specs=in_specs,
                out_specs=out_specs,
                check_rep=False,
            ),
            donate_argnums=donate,
            keep_unused=True,
        )

    sharding = NamedSharding(mesh, PartitionSpec("core"))
    return make_fn, in_names, out_names, zero_outs, sharding


def _get_runner(reps=1):
    key = f"runner{reps}"
    if key not in _NC_CACHE:
        _NC_CACHE[key] = _make_runner(reps)
    return _NC_CACHE[key]


LAST_BENCH = None


def kernel(**inputs):
    global LAST_BENCH
    import time

    import jax

    make_fn, in_names, out_names, zero_outs, sharding = _get_runner()
    in_maps = _host_prep(inputs)
    concat_in = [
        np.concatenate([np.asarray(in_maps[c][n]) for c in range(B)], axis=0)
        for n in in_names
    ]
    concat_zeros = [
        np.zeros((B * z.shape[0], *z.shape[1:]), z.dtype) for z in zero_outs
    ]
    fn1 = make_fn(1)
    dev_in = [jax.device_put(a, sharding) for a in concat_in]
    outs = fn1(*dev_in, *concat_zeros)
    jax.block_until_ready(outs)
    result = np.asarray(outs[0]).reshape(B, N, C).astype(np.float32)

    iters = int(os.environ.get("BENCH_ITERS", "0"))
    if iters > 0:
        o = fn1(*dev_in, *outs)  # warm
        jax.block_until_ready(o)
        times = []
        for _ in range(iters):
            t0 = time.perf_counter()
            o = fn1(*dev_in, *o)
            jax.block_until_ready(o)
            times.append(time.perf_counter() - t0)
        overhead = _bench_overhead()
        t_min = float(np.min(times))
        t_med = float(np.median(times))
        LAST_BENCH = {
            "per_iter_ns": max(t_min - overhead, 0.0) * 1e9,
            "call_min_ns": t_min * 1e9,
            "call_med_ns": t_med * 1e9,
            "overhead_ns": overhead * 1e9,
            "iters": iters,
        }
    return result


def _bench_overhead():
    """Per-call dispatch overhead, measured with a trivial 1-DMA kernel."""
    import time

    import jax
    from jax.experimental.shard_map import shard_map
    from jax.sharding import Mesh, PartitionSpec

    import concourse.bacc as bacc
    import concourse.tile as tile
    from concourse import bass2jax, mybir

    if "tiny" not in _NC_CACHE:
        f32 = mybir.dt.float32
        nc = bacc.Bacc(
            "TRN2",
            target_bir_lowering=False,
            debug=False,
            enable_asserts=False,
            num_devices=8,
        )
        xi = nc.dram_tensor("ti", [128, 128], f32, kind="ExternalInput").ap()
        xo = nc.dram_tensor("to", [128, 128], f32, kind="ExternalOutput").ap()
        with tile.TileContext(nc) as tc:
            with tc.tile_pool(name="p", bufs=1) as p:
                t = p.tile([128, 128], f32, tag="t", name="t")
                nc.sync.dma_start(t[:], xi[:])
                nc.sync.dma_start(xo[:], t[:])
        nc.compile()

        partition_name = nc.partition_id_tensor.name if nc.partition_id_tensor else None
        all_names = ["ti", "to"]
        if partition_name is not None:
            all_names.append(partition_name)
        out_avals = [jax.core.ShapedArray((128, 128), np.float32)]

        def _tbody(*args):
            operands = list(args)
            if partition_name is not None:
                operands.append(bass2jax.partition_id_tensor())
            return tuple(
                bass2jax._bass_exec_p.bind(
                    *operands,
                    out_avals=tuple(out_avals),
                    in_names=tuple(all_names),
                    out_names=("to",),
                    lowering_input_output_aliases=(),
                    sim_require_finite=True,
                    sim_require_nnan=True,
                    nc=nc,
                )
            )

        devices = jax.devices()[:B]
        mesh = Mesh(np.asarray(devices), ("core",))
        tfn = jax.jit(
            shard_map(
                _tbody,
                mesh=mesh,
                in_specs=(PartitionSpec("core"),) * 2,
                out_specs=(PartitionSpec("core"),),
                check_rep=False,
            ),
            donate_argnums=(1,),
            keep_unused=True,
        )
        _NC_CACHE["tiny"] = tfn

    tfn = _NC_CACHE["tiny"]
    ti = np.zeros((B * 128, 128), np.float32)
    o = tfn(ti, np.zeros((B * 128, 128), np.float32))
    jax.block_until_ready(o)
    times = []
    for _ in range(30):
        t0 = time.perf_counter()
        o = tfn(ti, *([o] if not isinstance(o, tuple) else list(o)))
        jax.block_until_ready(o)
        times.append(time.perf_counter() - t0)
    return float(np.min(times))



# revision 17
# speedup vs baseline: 1.4494x; 1.4494x over previous
"""Trainium2 Bass kernel for a dense transformer block (B=8, N=1024, C=768, H=12).

Sharding: data-parallel over batch -- one batch element per NeuronCore (8 cores),
weights replicated, no collectives.

Per-core dataflow (x_b: [1024, 768]):
  LN1 (token-major, gain folded into qkv_w on host) -> bf16 h1
  -> transpose to feature-major hT [128, 6x1024] fp8 (weights shipped x16 fp8)
  -> QKV via fp8 DoubleRow matmuls: q,k feature-major bf16 [64, 1024]/head
     (scale x16 kept, removed at exp via scale=1/256); v token-major bf16
     (+ ones column), descaled 1/16 at PSUM->SBUF copy
  -> per head: scores^T [keys, queries] = kT.T @ qT (bf16), exp with
     scale=1/256 (no max-sub; scores are small by construction), multiply by
     host-precomputed exp(rel-bias) Toeplitz band (bf16, DVE 4x mode),
     AV matmul with ones column -> unnormalized attn out + softmax sums
     (sums DMA'd straight out of PSUM); normalize via reciprocal (x16 folded
     into selector) + PE broadcast + multiply -> aT fp8 [128, 6x1024]
  -> proj via fp8 DoubleRow (1/256 descale at PSUM->SBUF) -> bf16 yT
  -> transpose -> +x residual (in-place f32) -> LN2 -> bf16 h2
  -> transpose -> h2T fp8 -> MLP fc1+gelu(scale 1/16)/fc2 via fp8 DoubleRow
  -> transpose -> +residual -> store.
"""

import os

import numpy as np

B, N, C, H, D = 8, 1024, 768, 12, 64
NT = N // 128   # 8 token tiles
KT = C // 128   # 6 feature tiles
F1 = 4 * C      # 3072
RT = F1 // 128  # 24
W = 2 * N - 1   # 2047 toeplitz band width
EPS = 1e-5
WS = 16.0       # fp8 weight scale

LAST_RESULTS = None  # stash of the last BassKernelResults (for test.py)

_NC_CACHE = {}


def _build_nc(reps=1):
    from contextlib import ExitStack

    import concourse.bacc as bacc
    import concourse.tile as tile
    from concourse import masks, mybir

    f32 = mybir.dt.float32
    f32r = mybir.dt.float32r
    bf16 = mybir.dt.bfloat16
    fp8 = mybir.dt.float8e4

    def R(ap):
        return ap.bitcast(f32r)
    AF = mybir.ActivationFunctionType
    AX = mybir.AxisListType
    OP = mybir.AluOpType
    DR = mybir.MatmulPerfMode.DoubleRow

    nc = bacc.Bacc(
        "TRN2",
        target_bir_lowering=False,
        debug=False,
        enable_asserts=False,
        num_devices=8,
    )

    x_d = nc.dram_tensor("x", [N, C], f32, kind="ExternalInput").ap()
    wqkv_d = nc.dram_tensor("wqkvdr", [3, 128, 2 * 3 * C], fp8, kind="ExternalInput").ap()
    bqkv_d = nc.dram_tensor("bqkv", [1, 3 * C], f32, kind="ExternalInput").ap()
    bv_d = nc.dram_tensor("bvrow", [1, C], bf16, kind="ExternalInput").ap()
    wproj_d = nc.dram_tensor("wprojdr", [3, 128, 2 * C], fp8, kind="ExternalInput").ap()
    bproj_d = nc.dram_tensor("bproj", [1, C], f32, kind="ExternalInput").ap()
    wfc1_d = nc.dram_tensor("wfc1t", [RT, 128, C], bf16, kind="ExternalInput").ap()
    bfc1_d = nc.dram_tensor("bfc1", [1, F1], f32, kind="ExternalInput").ap()
    wfc2_d = nc.dram_tensor("wfc2T", [F1, C], bf16, kind="ExternalInput").ap()
    bfc2_d = nc.dram_tensor("bfc2", [1, C], f32, kind="ExternalInput").ap()
    rb_d = nc.dram_tensor("eband", [H, 128, W], bf16, kind="ExternalInput").ap()
    sel_d = nc.dram_tensor("sel", [2, 128], f32, kind="ExternalInput").ap()
    out_d = nc.dram_tensor("out", [N, C], f32, kind="ExternalOutput").ap()

    with tile.TileContext(nc) as tc, ExitStack() as ctx:
        # ---------------- kernel-wide pools (opened first, closed last: LIFO ok)
        cpool = ctx.enter_context(tc.tile_pool(name="const", bufs=1))
        identb = cpool.tile([128, 128], bf16, tag="identb")
        masks.make_identity(nc, identb[:])
        onesP = cpool.tile([1, 128], bf16, tag="onesP")
        nc.any.memset(onesP[:], 1.0)
        sel = cpool.tile([2, 128], f32r, tag="sel")
        nc.sync.dma_start(sel[:], sel_d[:].bitcast(f32r))
        epsc = cpool.tile([128, 1], f32, tag="eps")
        nc.any.memset(epsc[:], EPS)
        bqkv_sb = cpool.tile([128, 18], f32, tag="bqkv")
        nc.sync.dma_start(bqkv_sb[:], bqkv_d[0].rearrange("(a p) -> p a", p=128))
        bv_row = cpool.tile([1, C], bf16, tag="bvrow")
        nc.sync.dma_start(bv_row[:], bv_d[:])
        bproj_sb = cpool.tile([128, 6], f32, tag="bproj")
        nc.sync.dma_start(bproj_sb[:], bproj_d[0].rearrange("(a p) -> p a", p=128))
        bfc1_sb = cpool.tile([128, RT], f32, tag="bfc1")
        nc.sync.dma_start(bfc1_sb[:], bfc1_d[0].rearrange("(a p) -> p a", p=128))
        bfc2_sb = cpool.tile([128, 6], f32, tag="bfc2")
        nc.sync.dma_start(bfc2_sb[:], bfc2_d[0].rearrange("(a p) -> p a", p=128))

        stat = ctx.enter_context(tc.tile_pool(name="stat", bufs=8))
        # chain pool: big persistent buffers
        chain = ctx.enter_context(tc.tile_pool(name="chain", bufs=1))

        def layernorm(dst_ap, src_ap, scratch_ap):
            """dst = (src - mean(src)) * rsqrt(var(src) + eps); scratch may alias dst."""
            sums = stat.tile([128, 1], f32, tag="sums", name="sums")
            nc.vector.reduce_sum(sums[:], src_ap, axis=AX.X)
            mu = stat.tile([128, 1], f32, tag="mu", name="mu")
            nc.vector.tensor_scalar_mul(mu[:], sums[:], 1.0 / C)
            nc.vector.tensor_mul(scratch_ap, src_ap, src_ap)
            ssq = stat.tile([128, 1], f32, tag="ssq", name="ssq")
            nc.vector.reduce_sum(ssq[:], scratch_ap, axis=AX.X)
            musq = stat.tile([128, 1], f32, tag="musq", name="musq")
            nc.vector.tensor_mul(musq[:], mu[:], mu[:])
            var = stat.tile([128, 1], f32, tag="var", name="var")
            nc.vector.tensor_scalar(
                var[:], ssq[:], 1.0 / C, musq[:], op0=OP.mult, op1=OP.subtract
            )
            sd = stat.tile([128, 1], f32, tag="sd", name="sd")
            nc.scalar.activation(sd[:], var[:], AF.Sqrt, bias=epsc[:])
            rstd = stat.tile([128, 1], f32, tag="rstd", name="rstd")
            nc.vector.reciprocal(rstd[:], sd[:])
            nmr = stat.tile([128, 1], f32, tag="nmr", name="nmr")
            nc.vector.tensor_scalar(
                nmr[:], mu[:], rstd[:], -1.0, op0=OP.mult, op1=OP.mult
            )
            nc.vector.tensor_scalar(
                dst_ap, src_ap, rstd[:], nmr[:], op0=OP.mult, op1=OP.add
            )

        for _rep in range(reps):
            # persistent per-batch state
            xs = [chain.tile([128, C], f32, tag="x", bufs=NT, name=f"x{t}") for t in range(NT)]
            hT = chain.tile([128, KT * N], fp8, tag="hT", bufs=1, name="hT")
            hTv = hT[:].rearrange("p (c q) -> p c q", q=N)
            vaug = [
                chain.tile([128, H * 65], bf16, tag="vaug", bufs=NT, name=f"vaug{t}")
                for t in range(NT)
            ]
            s_all = chain.tile([H, N], f32, tag="sall", bufs=1, name="sall")

            # ---------------- phase A+B: load x, LN1, transpose -> hT (fp8)
            with (
                tc.tile_pool(name="hlnp", bufs=4) as hlnp,
                tc.tile_pool(name="psB", bufs=4, space="PSUM") as psB,
            ):
                for t in range(NT):
                    nc.sync.dma_start(xs[t][:], x_d[t * 128 : (t + 1) * 128, :])
                    h1 = hlnp.tile([128, C], bf16, tag="hln", name=f"h1_{t}")
                    layernorm(h1[:], xs[t][:], h1[:])
                    for half, cw in ((0, 4), (1, 2)):
                        c0 = half * 4
                        ps = psB.tile([128, 512], bf16, tag="tp", name="psb")
                        for k in range(cw):
                            nc.tensor.transpose(
                                ps[:, k * 128 : (k + 1) * 128],
                                h1[:, (c0 + k) * 128 : (c0 + k + 1) * 128],
                                identb[:],
                            )
                        nc.vector.tensor_copy(
                            hTv[:, c0 : c0 + cw, t * 128 : (t + 1) * 128],
                            ps[:, : cw * 128].rearrange("p (c n) -> p c n", n=128),
                        )

            # ---------------- phase C: QKV (fp8 DoubleRow; weights x16)
            qkT = [
                chain.tile([128, N], bf16, tag="qkT", bufs=12, name=f"qkT{i}")
                for i in range(12)
            ]
            with tc.tile_pool(name="wqkv", bufs=3) as wq_pool:
                wq = []
                for i in range(3):
                    wt = wq_pool.tile([128, 2 * 3 * C], fp8, tag="wq", name=f"wq{i}")
                    nc.sync.dma_start(wt[:], wqkv_d[i])
                    wq.append(wt)
                wqv = [w[:].rearrange("p (s f) -> p s f", s=2) for w in wq]
                with tc.tile_pool(name="psC", bufs=3, space="PSUM") as psC:
                    # q,k feature-major
                    for jt in range(12):
                        for qc in range(2):
                            ps = psC.tile([128, 512], f32, tag="ps", name="psc")
                            for i in range(3):
                                nc.tensor.matmul(
                                    ps[:],
                                    wqv[i][:, :, jt * 128 : (jt + 1) * 128],
                                    hTv[:, 2 * i : 2 * i + 2, qc * 512 : (qc + 1) * 512],
                                    start=(i == 0),
                                    stop=(i == 2),
                                    perf_mode=DR,
                                )
                            nc.vector.tensor_scalar(
                                qkT[jt][:, qc * 512 : (qc + 1) * 512],
                                ps[:],
                                1.0 / WS,
                                bqkv_sb[:, jt : jt + 1],
                                op0=OP.mult,
                                op1=OP.add,
                            )
                    # v token-major (descale 1/16 at copy), ones col for sums
                    for t in range(NT):
                        vview = vaug[t][:].rearrange("p (h e) -> p h e", e=65)
                        for vc in range(2):
                            ps = psC.tile([128, 384], f32, tag="psv", bufs=2, name="psv")
                            for i in range(3):
                                nc.tensor.matmul(
                                    ps[:],
                                    hTv[:, 2 * i : 2 * i + 2, t * 128 : (t + 1) * 128],
                                    wqv[i][:, :, 2 * C + vc * 384 : 2 * C + (vc + 1) * 384],
                                    start=(i == 0),
                                    stop=False,
                                    perf_mode=DR,
                                )
                            nc.tensor.matmul(
                                ps[:],
                                onesP[:],
                                bv_row[:, vc * 384 : (vc + 1) * 384],
                                start=False,
                                stop=True,
                            )
                            nc.vector.tensor_scalar_mul(
                                vview[:, vc * 6 : (vc + 1) * 6, 0:64],
                                ps[:].rearrange("p (h e) -> p h e", e=64),
                                1.0 / WS,
                            )
                        nc.any.memset(vview[:, :, 64:65], 1.0)

            # ---------------- phase D: attention
            aT = chain.tile([128, KT * N], fp8, tag="aT", bufs=1, name="aT")
            aTv = aT[:].rearrange("p (c q) -> p c q", q=N)
            with (
                tc.tile_pool(name="rbp", bufs=4) as rbp,
                tc.tile_pool(name="ptp", bufs=18) as ptp,
                tc.tile_pool(name="srowp", bufs=4) as srowp,
                tc.tile_pool(name="oddp", bufs=3) as oddp,
                tc.tile_pool(name="psS", bufs=3, space="PSUM") as psS,
                tc.tile_pool(name="psAV", bufs=2, space="PSUM") as psAV,
            ):
                for hp in range(KT):
                    # head pair (2hp, 2hp+1): even head at partitions 0:64, odd at
                    # 64:128 of the same qkT tiles
                    rbs = []
                    for odd in range(2):
                        rb = rbp.tile([128, W], bf16, tag="rb", name=f"rb{2*hp+odd}")
                        nc.sync.dma_start(rb[:], rb_d[2 * hp + odd])
                        rbs.append(rb)
                    ptiles = [[], []]
                    for kc in range(NT):
                        for odd in range(2):
                            ro = odd * 64
                            ps = psS.tile([128, 1024], f32, tag="ps", name="pss")
                            for qc in range(2):
                                nc.tensor.matmul(
                                    ps[:, qc * 512 : (qc + 1) * 512],
                                    qkT[6 + hp][ro : ro + 64, kc * 128 : (kc + 1) * 128],
                                    qkT[hp][ro : ro + 64, qc * 512 : (qc + 1) * 512],
                                    start=True,
                                    stop=True,
                                )
                            pt = ptp.tile([128, 1024], bf16, tag="pt", name="pt")
                            nc.scalar.activation(pt[:], ps[:], AF.Exp)
                            off = 1023 - kc * 128
                            nc.vector.tensor_mul(
                                pt[:], pt[:], rbs[odd][:, off : off + 1024]
                            )
                            ptiles[odd].append(pt)
                    for odd in range(2):
                        h = 2 * hp + odd
                        for qc in range(2):
                            pav = psAV.tile([128, 512], f32, tag="pav", name="pav")
                            for kc in range(NT):
                                nc.tensor.matmul(
                                    pav[0:65, :],
                                    vaug[kc][:, h * 65 : (h + 1) * 65],
                                    ptiles[odd][kc][:, qc * 512 : (qc + 1) * 512],
                                    start=(kc == 0),
                                    stop=(kc == NT - 1),
                                )
                            if odd:
                                tmp = oddp.tile([128, 512], fp8, tag="odd", name="avodd")
                                nc.vector.tensor_copy(tmp[0:64, :], pav[0:64, :])
                                nc.sync.dma_start(
                                    aTv[64:128, hp, qc * 512 : (qc + 1) * 512],
                                    tmp[0:64, :],
                                )
                            else:
                                nc.vector.tensor_copy(
                                    aTv[0:64, hp, qc * 512 : (qc + 1) * 512],
                                    pav[0:64, :],
                                )
                            srow = srowp.tile([128, 512], f32, tag="srow", name="srow")
                            nc.vector.tensor_copy(srow[64:65, :], pav[64:65, :])
                            nc.sync.dma_start(
                                s_all[h : h + 1, qc * 512 : (qc + 1) * 512],
                                srow[64:65, :],
                            )

            # ---------------- normalize (x WS folded into sel) + proj (fp8 DR)
            yT = [
                chain.tile([128, N], bf16, tag="yT", bufs=KT, name=f"yT{i}")
                for i in range(KT)
            ]
            with tc.tile_pool(name="wpp", bufs=3) as wpp:
                wp = []
                for i in range(3):
                    wt = wpp.tile([128, 2 * C], fp8, tag="wp", name=f"wp{i}")
                    nc.sync.dma_start(wt[:], wproj_d[i])
                    wp.append(wt)
                wpv = [w[:].rearrange("p (s f) -> p s f", s=2) for w in wp]
                with (
                    tc.tile_pool(name="stgp", bufs=6) as stgp,
                    tc.tile_pool(name="psE", bufs=KT, space="PSUM") as psE,
                    tc.tile_pool(name="psNorm", bufs=2, space="PSUM") as psN,
                ):
                    nc.vector.reciprocal_approx_fast(s_all[:], s_all[:])
                    for qc in range(2):
                        for hp in range(KT):
                            st2 = stgp.tile([2, 512], f32r, tag="stg", name="st2")
                            nc.sync.dma_start(
                                st2[:],
                                s_all[
                                    2 * hp : 2 * hp + 2, qc * 512 : (qc + 1) * 512
                                ].bitcast(f32r),
                            )
                            psb = psN.tile([128, 512], f32, tag="psn", name="psn")
                            nc.tensor.matmul(
                                psb[:], sel[:], st2[:], start=True, stop=True
                            )
                            nc.vector.tensor_mul(
                                aTv[:, hp, qc * 512 : (qc + 1) * 512],
                                aTv[:, hp, qc * 512 : (qc + 1) * 512],
                                psb[:],
                            )
                        pse = [
                            psE.tile([128, 512], f32, tag="pse", name=f"pse{qc}_{co}")
                            for co in range(KT)
                        ]
                        for co in range(KT):
                            for i in range(3):
                                nc.tensor.matmul(
                                    pse[co][:],
                                    wpv[i][:, :, co * 128 : (co + 1) * 128],
                                    aTv[:, 2 * i : 2 * i + 2, qc * 512 : (qc + 1) * 512],
                                    start=(i == 0),
                                    stop=(i == 2),
                                    perf_mode=DR,
                                )
                            nc.vector.tensor_scalar(
                                yT[co][:, qc * 512 : (qc + 1) * 512],
                                pse[co][:],
                                1.0 / (WS * WS),
                                bproj_sb[:, co : co + 1],
                                op0=OP.mult,
                                op1=OP.add,
                            )

            # ---------------- phase F: transpose y, residual in-place, LN2 -> h2T
            h2T = chain.tile([128, KT * N], bf16, tag="h2T", bufs=1, name="h2T")
            h2Tv = h2T[:].rearrange("p (c q) -> p c q", q=N)
            with (
                tc.tile_pool(name="hlnp2", bufs=4) as hlnp2,
                tc.tile_pool(name="psF", bufs=4, space="PSUM") as psF,
            ):
                for t in range(NT):
                    for half, cw in ((0, 4), (1, 2)):
                        c0 = half * 4
                        ps = psF.tile([128, 512], bf16, tag="tp", name="psf")
                        for k in range(cw):
                            nc.tensor.transpose(
                                ps[:, k * 128 : (k + 1) * 128],
                                yT[c0 + k][:, t * 128 : (t + 1) * 128],
                                identb[:],
                            )
                        nc.vector.tensor_add(
                            xs[t][:, c0 * 128 : (c0 + cw) * 128],
                            xs[t][:, c0 * 128 : (c0 + cw) * 128],
                            ps[:, : cw * 128],
                        )
                    h2 = hlnp2.tile([128, C], bf16, tag="hln2", name=f"h2_{t}")
                    layernorm(h2[:], xs[t][:], h2[:])
                    for half, cw in ((0, 4), (1, 2)):
                        c0 = half * 4
                        ps = psF.tile([128, 512], bf16, tag="tp2", name="psf2")
                        for k in range(cw):
                            nc.tensor.transpose(
                                ps[:, k * 128 : (k + 1) * 128],
                                h2[:, (c0 + k) * 128 : (c0 + k + 1) * 128],
                                identb[:],
                            )
                        nc.vector.tensor_copy(
                            h2Tv[:, c0 : c0 + cw, t * 128 : (t + 1) * 128],
                            ps[:, : cw * 128].rearrange("p (c n) -> p c n", n=128),
                        )

            # ---------------- phase H: MLP (fp8 DR) + final residual + store
            with (
                tc.tile_pool(name="w1p", bufs=4) as w1p,
                tc.tile_pool(name="w2p", bufs=3) as w2p,
                tc.tile_pool(name="grp", bufs=3) as grp,
                tc.tile_pool(name="o2p", bufs=7) as o2p,
                tc.tile_pool(name="obp", bufs=3) as obp,
            ):
                for qc in range(2):
                    with tc.tile_pool(name="psO", bufs=6, space="PSUM") as ps_o:
                        pso = [
                            ps_o.tile([128, 512], f32, tag="pso", name=f"pso{qc}_{i}")
                            for i in range(KT)
                        ]
                        with tc.tile_pool(name="psG2", bufs=2, space="PSUM") as ps_g:
                            for r in range(RT):
                                w1 = w1p.tile([128, C], bf16, tag="w1", name=f"w1_{r}")
                                nc.sync.dma_start(w1[:], wfc1_d[r])
                                w2 = w2p.tile([128, C], bf16, tag="w2", name=f"w2_{r}")
                                nc.sync.dma_start(
                                    w2[:], wfc2_d[r * 128 : (r + 1) * 128, :]
                                )
                                psg = ps_g.tile([128, 512], f32, tag="psg", name="psg")
                                for ct in range(KT):
                                    nc.tensor.matmul(
                                        psg[:],
                                        w1[:, ct * 128 : (ct + 1) * 128],
                                        h2Tv[:, ct, qc * 512 : (qc + 1) * 512],
                                        start=(ct == 0),
                                        stop=(ct == KT - 1),
                                    )
                                gr = grp.tile([128, 512], bf16, tag="gr", name="gr")
                                nc.scalar.activation(
                                    gr[:], psg[:], AF.Gelu, bias=bfc1_sb[:, r : r + 1]
                                )
                                for co in range(KT):
                                    nc.tensor.matmul(
                                        pso[co][:],
                                        w2[:, co * 128 : (co + 1) * 128],
                                        gr[:],
                                        start=(r == 0),
                                        stop=(r == RT - 1),
                                    )
                        o2 = []
                        for co in range(KT):
                            o2t = o2p.tile([128, 512], bf16, tag="o2", name=f"o2_{qc}_{co}")
                            nc.vector.tensor_scalar_add(
                                o2t[:], pso[co][:], bfc2_sb[:, co : co + 1]
                            )
                            o2.append(o2t)
                    with tc.tile_pool(name="psH", bufs=4, space="PSUM") as psH:
                        for t4 in range(4):
                            t = qc * 4 + t4
                            ob = obp.tile([128, C], f32, tag="ob", name="ob")
                            for half, cw in ((0, 4), (1, 2)):
                                c0 = half * 4
                                ps = psH.tile([128, 512], bf16, tag="tp", name="psh")
                                for k in range(cw):
                                    nc.tensor.transpose(
                                        ps[:, k * 128 : (k + 1) * 128],
                                        o2[c0 + k][:, t4 * 128 : (t4 + 1) * 128],
                                        identb[:],
                                    )
                                nc.vector.tensor_add(
                                    ob[:, c0 * 128 : (c0 + cw) * 128],
                                    xs[t][:, c0 * 128 : (c0 + cw) * 128],
                                    ps[:, : cw * 128],
                                )
                            nc.sync.dma_start(out_d[t * 128 : (t + 1) * 128, :], ob[:])

    nc.compile()
    return nc


def _get_nc(reps=1):
    key = f"nc{reps}"
    if key not in _NC_CACHE:
        _NC_CACHE[key] = _build_nc(reps)
    return _NC_CACHE[key]


def _host_prep(inputs):
    import ml_dtypes

    fp8 = ml_dtypes.float8_e4m3
    bf16 = ml_dtypes.bfloat16

    inp = {k: np.asarray(v) for k, v in inputs.items()}
    x = np.ascontiguousarray(inp["x"], dtype=np.float32)  # [8, 1024, 768]
    g1 = inp["ln1_g"].astype(np.float64)
    b1 = inp["ln1_b"].astype(np.float64)
    qkv_w = inp["qkv_w"].astype(np.float64)  # [2304, 768]
    Ws = qkv_w.copy()
    Ws[:C] *= D ** (-0.5)  # fold attention scale into Wq
    wqkvT = (Ws * g1[None, :]).T * WS  # [768, 2304]
    # DR pairs: wqkv_dr[i, p, s, f] = wqkvT[(2i+s)*128+p, f]
    wqkv_dr = np.ascontiguousarray(
        wqkvT.reshape(3, 2, 128, 3 * C).transpose(0, 2, 1, 3).reshape(3, 128, 2 * 3 * C)
    ).astype(fp8)
    bqkv = (Ws @ b1).astype(np.float32).reshape(1, 3 * C)
    bv = (Ws[2 * C :] @ b1 * WS).astype(bf16).reshape(1, C)

    wprojT = inp["proj_w"].astype(np.float64).T * WS  # [768, 768]
    wproj_dr = np.ascontiguousarray(
        wprojT.reshape(3, 2, 128, C).transpose(0, 2, 1, 3).reshape(3, 128, 2 * C)
    ).astype(fp8)
    bproj = inp["proj_b"].astype(np.float32).reshape(1, C)

    g2 = inp["ln2_g"].astype(np.float64)
    b2 = inp["ln2_b"].astype(np.float64)
    fc1_w = inp["fc1_w"].astype(np.float64)  # [3072, 768]
    wfc1T = (fc1_w * g2[None, :]).T  # [768, 3072]
    # pre-tiled: wfc1t[r, p, ct*128+j] = wfc1T[ct*128+p, r*128+j]
    wfc1t = np.ascontiguousarray(
        wfc1T.reshape(KT, 128, RT, 128).transpose(2, 1, 0, 3).reshape(RT, 128, C)
    ).astype(bf16)
    bfc1 = (fc1_w @ b2 + inp["fc1_b"].astype(np.float64)).astype(np.float32)
    bfc1 = bfc1.reshape(1, F1)
    wfc2T = np.ascontiguousarray(inp["fc2_w"].astype(np.float64).T).astype(bf16)
    bfc2 = inp["fc2_b"].astype(np.float32).reshape(1, C)

    # rel-bias toeplitz band of exp(bias):
    # eband[h, p, w] = exp(rel_table[clip(p + 1087 - w, 0, 128), h])
    tab = inp["rel_table"].astype(np.float64)  # [129, 12]
    p_i = np.arange(128)
    w_i = np.arange(W)
    idx = np.clip(p_i[:, None] + (N + 63) - w_i[None, :], 0, 2 * 64)
    eband = np.ascontiguousarray(
        np.exp(tab)[idx, :].transpose(2, 0, 1)
    ).astype(bf16)  # [12, 128, 2047]

    sel = np.zeros((2, 128), np.float32)
    sel[0, 0:64] = WS
    sel[1, 64:128] = WS
    shared = {
        "sel": sel,
        "wqkvdr": wqkv_dr,
        "bqkv": bqkv,
        "bvrow": bv,
        "wprojdr": wproj_dr,
        "bproj": bproj,
        "wfc1t": wfc1t,
        "bfc1": bfc1,
        "wfc2T": wfc2T,
        "bfc2": bfc2,
        "eband": eband,
    }
    in_maps = [{"x": np.ascontiguousarray(x[c]), **shared} for c in range(B)]
    return in_maps


def _make_runner(reps=1):
    import jax
    from jax.experimental.shard_map import shard_map
    from jax.sharding import Mesh, NamedSharding, PartitionSpec

    from concourse import bass2jax, mybir

    nc = _get_nc(reps)
    bass2jax.install_neuronx_cc_hook()

    partition_name = nc.partition_id_tensor.name if nc.partition_id_tensor else None
    in_names, out_names, out_avals, zero_outs = [], [], [], []
    for alloc in nc.m.functions[0].allocations:
        if not isinstance(alloc, mybir.MemoryLocationSet):
            continue
        name = alloc.memorylocations[0].name
        if alloc.kind == "ExternalInput":
            if name != partition_name:
                in_names.append(name)
        elif alloc.kind == "ExternalOutput":
            out_names.append(name)
            shape = tuple(alloc.tensor_shape)
            dtype = mybir.dt.np(alloc.dtype)
            out_avals.append(jax.core.ShapedArray(shape, dtype))
            zero_outs.append(np.zeros(shape, dtype))
    n_params = len(in_names)
    all_names = tuple(in_names) + tuple(out_names)
    if partition_name is not None:
        all_names = all_names + (partition_name,)
    donate = tuple(range(n_params, n_params + len(out_names)))

    def _body(*args):
        operands = list(args)
        if partition_name is not None:
            operands.append(bass2jax.partition_id_tensor())
        outs = bass2jax._bass_exec_p.bind(
            *operands,
            out_avals=tuple(out_avals),
            in_names=all_names,
            out_names=tuple(out_names),
            lowering_input_output_aliases=(),
            sim_require_finite=True,
            sim_require_nnan=True,
            nc=nc,
        )
        return tuple(outs)

    def _body_k(k):
        def body(*args):
            ins = list(args[:n_params])
            outs = list(args[n_params:])
            for _ in range(k):
                outs = list(_body(*ins, *outs))
            return tuple(outs)

        return body

    devices = jax.devices()[:B]
    mesh = Mesh(np.asarray(devices), ("core",))
    in_specs = (PartitionSpec("core"),) * (n_params + len(out_names))
    out_specs = (PartitionSpec("core"),) * len(out_names)

    def make_fn(k):
        return jax.jit(
            shard_map(
                _body_k(k),
                mesh=mesh,
                in_specs=in_specs,
                out_specs=out_specs,
                check_rep=False,
            ),
            donate_argnums=donate,
            keep_unused=True,
        )

    sharding = NamedSharding(mesh, PartitionSpec("core"))
    return make_fn, in_names, out_names, zero_outs, sharding


def _get_runner(reps=1):
    key = f"runner{reps}"
    if key not in _NC_CACHE:
        _NC_CACHE[key] = _make_runner(reps)
    return _NC_CACHE[key]


LAST_BENCH = None


def kernel(**inputs):
    global LAST_BENCH
    import time

    import jax

    make_fn, in_names, out_names, zero_outs, sharding = _get_runner()
    in_maps = _host_prep(inputs)
    concat_in = [
        np.concatenate([np.asarray(in_maps[c][n]) for c in range(B)], axis=0)
        for n in in_names
    ]
    concat_zeros = [
        np.zeros((B * z.shape[0], *z.shape[1:]), z.dtype) for z in zero_outs
    ]
    fn1 = make_fn(1)
    dev_in = [jax.device_put(a, sharding) for a in concat_in]
    outs = fn1(*dev_in, *concat_zeros)
    jax.block_until_ready(outs)
    result = np.asarray(outs[0]).reshape(B, N, C).astype(np.float32)

    iters = int(os.environ.get("BENCH_ITERS", "0"))
    if iters > 0:
        o = fn1(*dev_in, *outs)  # warm
        jax.block_until_ready(o)
        times = []
        for _ in range(iters):
            t0 = time.perf_counter()
            o = fn1(*dev_in, *o)
            jax.block_until_ready(o)
            times.append(time.perf_counter() - t0)
        overhead = _bench_overhead()
        t_min = float(np.min(times))
        t_med = float(np.median(times))
        LAST_BENCH = {
            "per_iter_ns": max(t_min - overhead, 0.0) * 1e9,
            "call_min_ns": t_min * 1e9,
            "call_med_ns": t_med * 1e9,
            "overhead_ns": overhead * 1e9,
            "iters": iters,
        }
    return result


def _bench_overhead():
    """Per-call dispatch overhead, measured with a trivial 1-DMA kernel."""
    import time

    import jax
    from jax.experimental.shard_map import shard_map
    from jax.sharding import Mesh, PartitionSpec

    import concourse.bacc as bacc
    import concourse.tile as tile
    from concourse import bass2jax, mybir

    if "tiny" not in _NC_CACHE:
        f32 = mybir.dt.float32
        nc = bacc.Bacc(
            "TRN2",
            target_bir_lowering=False,
            debug=False,
            enable_asserts=False,
            num_devices=8,
        )
        xi = nc.dram_tensor("ti", [128, 128], f32, kind="ExternalInput").ap()
        xo = nc.dram_tensor("to", [128, 128], f32, kind="ExternalOutput").ap()
        with tile.TileContext(nc) as tc:
            with tc.tile_pool(name="p", bufs=1) as p:
                t = p.tile([128, 128], f32, tag="t", name="t")
                nc.sync.dma_start(t[:], xi[:])
                nc.sync.dma_start(xo[:], t[:])
        nc.compile()

        partition_name = nc.partition_id_tensor.name if nc.partition_id_tensor else None
        all_names = ["ti", "to"]
        if partition_name is not None:
            all_names.append(partition_name)
        out_avals = [jax.core.ShapedArray((128, 128), np.float32)]

        def _tbody(*args):
            operands = list(args)
            if partition_name is not None:
                operands.append(bass2jax.partition_id_tensor())
            return tuple(
                bass2jax._bass_exec_p.bind(
                    *operands,
                    out_avals=tuple(out_avals),
                    in_names=tuple(all_names),
                    out_names=("to",),
                    lowering_input_output_aliases=(),
                    sim_require_finite=True,
                    sim_require_nnan=True,
                    nc=nc,
                )
            )

        devices = jax.devices()[:B]
        mesh = Mesh(np.asarray(devices), ("core",))
        tfn = jax.jit(
            shard_map(
                _tbody,
                mesh=mesh,
                in_specs=(PartitionSpec("core"),) * 2,
                out_specs=(PartitionSpec("core"),),
                check_rep=False,
            ),
            donate_argnums=(1,),
            keep_unused=True,
        )
        _NC_CACHE["tiny"] = tfn

    tfn = _NC_CACHE["tiny"]
    ti = np.zeros((B * 128, 128), np.float32)
    o = tfn(ti, np.zeros((B * 128, 128), np.float32))
    jax.block_until_ready(o)
    times = []
    for _ in range(30):
        t0 = time.perf_counter()
        o = tfn(ti, *([o] if not isinstance(o, tuple) else list(o)))
        jax.block_until_ready(o)
        times.append(time.perf_counter() - t0)
    return float(np.min(times))
